# revision 66
# baseline (speedup 1.0000x reference)
"""DMPNN layer kernel for 8 Trainium2 NeuronCores.

Sharding: data-parallel over destination nodes j (dim 2 of edge_attr/adj,
dim 1 of the output). Each core gets a 64-column j-slice of edge_attr/adj,
the full h (needed because messages sum over all source nodes i), and the
small weights replicated. The batch-global mask (adj.sum(0) > 0) only needs
the core's own j-slice of adj over the full batch, so no collective at all.

Math per core (j in its 64-column slice, source nodes i = 4p + q):
  mask[i,j]   = max_b adj[b,i,j]                    (adj is 0/1)
  deg[j]      = sum_i mask[i,j]
  mh[b,j,f]   = sum_i mask[i,j] h[b,i,f]
  me[b,j,e]   = sum_i mask[i,j] edge[b,i,j,e]
  out[b,j,o]  = sum_k G[o,k] x[b,k,j] + sum_f U[o,f] h[b,j,f] + ub[o]
     where G = U @ [Wh | wb | We]  (fused on device) and
     x[b] = [mh[b]^T; deg; me[b]^T]  (73 rows).

Design notes (measured on this part): PE ~1.2 GHz with ~165ns/instr
overhead; DVE ~1 elem/cyc/lane fp32 with ~0.3-0.4us/op overhead and big
penalties for broadcast/strided APs; dma_start occupies its issuing engine
~0.7-0.9us; HWDGE ~330 GB/s per queue; HBM ~358 GB/s/core.
 - sync queue: ident, adjA, h, hs, then the 8x1MiB edge stream.
   scalar queue: weights, adjB, then per-batch me moves + out stores.
 - mask via 7-op pairwise max tree, then pre-expanded over e to a
   contiguous f32 [128, q*j*e] tile so the per-batch edge multiply has
   no broadcast AP (the broadcast costs ~1.5x on DVE).
 - mh for ALL batches via 4 accumulating matmuls (mask chunk stationary,
   h for all 8 batches as a 512-wide rhs), then one [64,64] PE transpose
   + copy per batch into the stacked rhs; deg via a ones-lhsT matmul.
 - per batch: contiguous DVE multiply -> contiguous half-fold (DVE) ->
   second fold on gpsimd -> ONE accumulating e-major matmul lands me^T
   flat; ONE matmul pair [U^T;ub]x[hsT;1] + G^T x mhTd accumulates the
   final output directly (msg is never materialized).
"""

import numpy as np


def _ensure_path():
    try:
        import concourse.bass  # noqa: F401
    except ImportError:
        import sys

        for p in ("/opt/trn_rl_repo", "/root/.axon_site/_ro/trn_rl_repo"):
            if p not in sys.path:
                sys.path.insert(0, p)


B, N, H, E = 8, 512, 64, 8
NCORES = 8
JB = N // NCORES  # 64 destination columns per core
CH = N // 128  # 4 source-node sub-chunks (i = 4p + q)


_CACHE = {}


def _build_program():
    _ensure_path()
    import concourse.bacc as bacc
    import concourse.mybir as mybir
    import concourse.tile as tile

    dt = mybir.dt
    f32 = dt.float32
    bf16 = dt.bfloat16
    i32 = dt.int32
    Alu = mybir.AluOpType

    import ml_dtypes

    nc = bacc.Bacc("TRN2", debug=False, num_devices=NCORES)

    i8 = dt.int8
    # edge/h/hs are pre-cast to bf16 on the host: the kernel's first op on
    # them is a bf16-rounding multiply/matmul anyway (mask is 0/1), so the
    # result is bit-identical while the HBM stream halves to ~4.9 MiB.
    # adj holds 0/1 -> int8. All are pre-arranged per-partition-contiguous
    # (1KB-run descriptors measured only ~75 GB/s; 4-8KB runs go full rate).
    edge = nc.dram_tensor("edge", [B, N, JB, E], bf16, kind="ExternalInput").ap()
    adjs = nc.dram_tensor(
        "adjs", [128, B * (N // 128) * JB], i8, kind="ExternalInput"
    ).ap()
    h = nc.dram_tensor(
        "h", [128, (N // 128) * B * H], bf16, kind="ExternalInput"
    ).ap()
    hs = nc.dram_tensor("hs", [JB, B * H], bf16, kind="ExternalInput").ap()
    # weights host-cast to bf16 (they are rounded to bf16 on-chip anyway)
    Ww = nc.dram_tensor("Ww", [H, H + E], bf16, kind="ExternalInput").ap()
    Wb = nc.dram_tensor("Wb", [1, H], bf16, kind="ExternalInput").ap()
    Uw = nc.dram_tensor("Uw", [H, H], bf16, kind="ExternalInput").ap()
    Ub = nc.dram_tensor("Ub", [1, H], bf16, kind="ExternalInput").ap()
    out = nc.dram_tensor("out", [B, H, JB], f32, kind="ExternalOutput").ap()

    ident_d = nc.inline_tensor(
        np.eye(64).astype(ml_dtypes.bfloat16), "ident"
    )

    KM = H + 1 + E  # 73 contraction rows of the fused message matmul
    KU = H + 1  # 65 contraction rows of the base output matmul
    QJ = CH * JB  # 256
    QJE = CH * JB * E  # 2048
    JE = JB * E  # 512

    with tile.TileContext(nc) as tc:
        with (
            tc.tile_pool(name="const", bufs=1) as cpool,
            tc.tile_pool(name="masked", bufs=3) as mpool,
            tc.tile_pool(name="acc", bufs=3) as apool,
            tc.tile_pool(name="small", bufs=4) as spool,
            tc.tile_pool(name="pe", bufs=2, space="PSUM") as ppool_e,
            tc.tile_pool(name="pmh", bufs=1, space="PSUM") as ppool_mh,
            tc.tile_pool(name="ptr", bufs=2, space="PSUM") as ppool_tr,
            tc.tile_pool(name="pbase", bufs=2, space="PSUM") as ppool_base,
        ):
            # ---------------- DMA issue plan ----------------
            # All small/strided inputs must land BEFORE the edge stream:
            # the 8 KiB-descriptor edge DMAs starve any concurrent queue
            # down to ~1/30th bandwidth share.
            # sync queue: adjA(b0..3), h, then the edge stream.
            # scalar queue: adjB, ident, weights, hs, then per-batch smalls.
            adj_sb = cpool.tile([128, B * QJ], i8)
            nc.sync.dma_start(out=adj_sb[:, 0 : 4 * QJ], in_=adjs[:, 0 : 4 * QJ])
            nc.scalar.dma_start(out=adj_sb[:, 4 * QJ :], in_=adjs[:, 4 * QJ :])

            ident_bf = cpool.tile([64, 64], bf16)
            nc.scalar.dma_start(out=ident_bf[:, :], in_=ident_d.ap()[:, :])
            Ww_sb = cpool.tile([H, H + E], bf16)
            nc.scalar.dma_start(out=Ww_sb[:, :], in_=Ww[:, :])
            Uw_sb = cpool.tile([H, H], bf16)
            nc.scalar.dma_start(out=Uw_sb[:, :], in_=Uw[:, :])
            wb_sb = cpool.tile([1, H], bf16)
            nc.scalar.dma_start(out=wb_sb[:, :], in_=Wb[:, :])
            ub_sb = cpool.tile([1, H], bf16)
            nc.scalar.dma_start(out=ub_sb[:, :], in_=Ub[:, :])
            hs_all = cpool.tile([JB, B * H], bf16)
            nc.scalar.dma_start(out=hs_all[:, :], in_=hs[:, :])

            # h arrives pre-cast and already q-major: [p, (q b f)]
            h_bf2 = cpool.tile([128, B * CH * H], bf16)
            nc.sync.dma_start(out=h_bf2[:, :], in_=h[:, :])

            # the 4 MiB bf16 edge stream on sync; contiguous 4 KiB/partition
            edge_t = [
                cpool.tile([128, QJE], bf16, name=f"edge{b}") for b in range(B)
            ]
            EH = QJE // 2
            for b in range(B):
                src = edge[b].rearrange("(p q) j e -> p (q j e)", q=CH)
                if b == B - 1:
                    # halves: the tail multiply starts on the first half
                    nc.sync.dma_start(out=edge_t[b][:, 0:EH], in_=src[:, 0:EH])
                    nc.sync.dma_start(out=edge_t[b][:, EH:], in_=src[:, EH:])
                else:
                    nc.sync.dma_start(out=edge_t[b][:, :], in_=src)

            # ---- constants ----
            ones_bf = cpool.tile([128, 1], bf16)
            nc.vector.memset(ones_bf[:, :], 1.0)

            # ---- mask: 3-op pairwise max tree on contiguous halves; the
            # two big levels run as bitwise-OR on int32-packed int8 (0/1
            # values), quartering the DVE element count ----
            adj32 = adj_sb[:, :].bitcast(i32)  # [128, 512] packed words
            mt0 = cpool.tile([128, QJ], i32, name="mt0")
            nc.vector.tensor_tensor(
                mt0[:, :], adj32[:, 0:QJ], adj32[:, QJ:], Alu.bitwise_or
            )
            mt1 = cpool.tile([128, QJ // 2], i32, name="mt1")
            nc.vector.tensor_tensor(
                mt1[:, :], mt0[:, 0 : QJ // 2], mt0[:, QJ // 2 :], Alu.bitwise_or
            )
            mt1_8 = mt1[:, :].bitcast(i8)  # [128, 2*QJ] bytes
            mask_f = cpool.tile([128, QJ], f32)
            nc.vector.tensor_tensor(
                mask_f[:, :], mt1_8[:, 0:QJ], mt1_8[:, QJ:], Alu.max
            )
            mask_bf = cpool.tile([128, QJ], bf16)
            nc.vector.tensor_copy(mask_bf[:, :], mask_f[:, :])

            # pre-expanded bf16 mask over e: contiguous per-batch multiply
            mask_x = cpool.tile([128, QJE], bf16)
            nc.vector.tensor_copy(
                mask_x.rearrange("p (q j e) -> p q j e", q=CH, j=JB),
                mask_f.rearrange("p (q j) -> p q j", q=CH).broadcast_to(
                    [128, CH, JB, E]
                ),
            )


            # ---- stationary operands built on-chip (no DMA moves) ----
            A_bf = cpool.tile([H, KM], bf16)  # [Wh | wb | We] (cols)
            UUb = cpool.tile([KU, H], bf16)  # [U^T; ub]
            GT = cpool.tile([KM, H], bf16)  # (U @ A)^T
            hsT_all = cpool.tile([KU, B * JB], bf16)  # [hsT; ones] per b
            nc.vector.memset(hsT_all[H : H + 1, :], 1.0)
            # stacked rhs for all batches: [mhT; deg; me^T], b-major cols
            mhTd = cpool.tile([KM, B * JB], bf16)

            def emit_prep1():
                # A = [Wh | wb | We]
                pwb = ppool_tr.tile([H, 1], bf16, tag="t", name="pwb")
                nc.tensor.transpose(pwb[:, :], wb_sb[:, :], ident_bf[0:1, 0:1])
                puw = ppool_tr.tile([H, H], bf16, tag="t", name="puw")
                nc.tensor.transpose(puw[:, :], Uw_sb[:, :], ident_bf[0:H, 0:H])
                nc.scalar.copy(A_bf[:, 0:H], Ww_sb[:, 0:H])
                nc.scalar.copy(A_bf[:, H + 1 : KM], Ww_sb[:, H : H + E])
                nc.scalar.copy(A_bf[:, H : H + 1], pwb[:, :])
                nc.scalar.copy(UUb[0:H, :], puw[:, :])
                nc.scalar.copy(UUb[H : H + 1, :], ub_sb[:, :])

                # hsT per batch via PE transposes (hs arrives bf16)
                for b in range(B):
                    pht = ppool_tr.tile([H, JB], bf16, tag="t", name="pht")
                    nc.tensor.transpose(
                        pht[:, :], hs_all[:, b * H : (b + 1) * H], ident_bf[:, :]
                    )
                    nc.scalar.copy(hsT_all[0:H, b * JB : (b + 1) * JB], pht[:, :])

                # deg row matmul (folds emitted after s1(0) on DVE)
                pdeg = ppool_e.tile([1, QJ], f32, tag="e", name="pdeg")
                nc.tensor.matmul(
                    pdeg[:, :], lhsT=ones_bf[:, :], rhs=mask_bf[:, :],
                    start=True, stop=True,
                )

                # GT = (U @ A)^T = A^T U^T : lhsT=A [o,k], rhs=U^T [o,o']
                pg = ppool_mh.tile([KM, H], f32, name="pg")
                nc.tensor.matmul(
                    pg[:, :], lhsT=A_bf[:, :], rhs=UUb[0:H, :],
                    start=True, stop=True,
                )
                nc.scalar.copy(GT[:, :], pg[:, :])
                return pdeg

            def emit_deg(pdeg):
                # deg row: psum fold x2 + broadcast into mhTd row H
                # (all on the otherwise-idle gpsimd engine)
                dg0 = spool.tile([1, QJ], f32, name="dg0")
                nc.scalar.copy(dg0[:, :], pdeg[:, :])
                dg1 = spool.tile([1, 2 * JB], f32, name="dg1")
                nc.gpsimd.tensor_tensor(
                    dg1[:, :], dg0[:, 0 : 2 * JB], dg0[:, 2 * JB : QJ], Alu.add
                )
                dg2 = spool.tile([1, JB], bf16, name="dg2")
                nc.gpsimd.tensor_tensor(
                    dg2[:, :], dg1[:, 0:JB], dg1[:, JB : 2 * JB], Alu.add
                )
                nc.gpsimd.tensor_copy(
                    mhTd[H : H + 1, :].rearrange("p (b j) -> p b j", b=B),
                    dg2.rearrange("p (o j) -> p o j", o=1).broadcast_to(
                        [1, B, JB]
                    ),
                )

            def emit_mh_prep():
                # mh for ALL batches: mask chunk stationary, h 512-wide rhs
                pmh = ppool_mh.tile([JB, B * H], f32, name="pmh")
                for c in range(CH):
                    nc.tensor.matmul(
                        pmh[:, :],
                        lhsT=mask_bf[:, c * JB : (c + 1) * JB],
                        rhs=h_bf2[:, c * B * H : (c + 1) * B * H],
                        start=(c == 0),
                        stop=(c == CH - 1),
                    )
                mh_sb = cpool.tile([JB, B * H], bf16, name="mh_sb")
                nc.scalar.copy(mh_sb[:, :], pmh[:, :])



                # mhT per batch via PE transposes into the stacked rhs
                for b in range(B):
                    pmt = ppool_tr.tile([JB, H], bf16, tag="t", name="pmt")
                    nc.tensor.transpose(
                        pmt[:, :], mh_sb[:, b * H : (b + 1) * H], ident_bf[:, :]
                    )
                    nc.scalar.copy(mhTd[0:H, b * JB : (b + 1) * JB], pmt[:, :])

            # ---------------- per-batch software pipeline ----------------
            st = [dict() for _ in range(B)]

            def s1_mult(b):
                masked = mpool.tile([128, QJE], bf16, name="masked")
                # contiguous multiply (pre-expanded mask, no broadcast AP);
                # the tail batch multiplies per half-DMA to overlap arrival
                if b == B - 1:
                    for half in range(2):
                        nc.vector.tensor_tensor(
                            out=masked[:, half * EH : (half + 1) * EH],
                            in0=edge_t[b][:, half * EH : (half + 1) * EH],
                            in1=mask_x[:, half * EH : (half + 1) * EH],
                            op=Alu.mult,
                        )
                else:
                    nc.vector.tensor_tensor(
                        out=masked[:, :], in0=edge_t[b][:, :], in1=mask_x[:, :],
                        op=Alu.mult,
                    )
                st[b]["masked"] = masked

            def s1_fold(b):
                masked = st[b]["masked"]
                psum_e = ppool_e.tile([1, JE], f32, tag="e", name="psum_e")
                # contiguous half-fold: (q0+q2 | q1+q3)
                acc = apool.tile([128, EH], bf16, name="acc")
                nc.vector.tensor_tensor(
                    out=acc[:, :], in0=masked[:, 0:EH], in1=masked[:, EH:],
                    op=Alu.add,
                )
                # alternate fold depth to balance DVE vs PE; the tail batch
                # takes the short-DVE-chain path
                if b % 2 == 0 or b == B - 1:
                    asum = apool.tile([128, JE], bf16, name="asum")
                    nc.vector.tensor_tensor(
                        out=asum[:, :], in0=acc[:, 0:JE], in1=acc[:, JE:],
                        op=Alu.add,
                    )
                    nc.tensor.matmul(
                        psum_e[:, :],
                        lhsT=ones_bf[:, :],
                        rhs=asum.rearrange("p (j e) -> p e j", e=E),
                        start=True, stop=True,
                    )
                else:
                    for half in range(2):
                        nc.tensor.matmul(
                            psum_e[:, :],
                            lhsT=ones_bf[:, :],
                            rhs=acc[:, half * JE : (half + 1) * JE].rearrange(
                                "p (j e) -> p e j", e=E
                            ),
                            start=(half == 0),
                            stop=(half == 1),
                        )
                st[b]["psum_e"] = psum_e

            def s1(b):
                s1_mult(b)
                s1_fold(b)

            # dedicated output staging (stores ride the sync queue, which
            # drains behind the edge stream -- fine, they are terminal)
            out_t = [spool.tile([H, JB], f32, name=f"out{b}") for b in range(B)]

            def s2(b):
                # me^T rows into the stacked rhs (8-descriptor move; SWDGE
                # for steady-state, HWDGE for the latency-critical tail)
                d = st[b]
                me_sb = spool.tile([1, JE], bf16, name="me_sb")
                nc.scalar.copy(out=me_sb[:, :], in_=d["psum_e"][:, :])
                eng = nc.scalar if b == B - 1 else nc.gpsimd
                eng.dma_start(
                    out=mhTd[H + 1 : KM, b * JB : (b + 1) * JB],
                    in_=me_sb.rearrange("p (e j) -> p e j", e=E),
                )

            def s3(b):
                # out^T[b] = U^T-part (hsT) + G^T-part (mhTd), one psum
                psum_o = ppool_base.tile([H, JB], f32, name="psum_o")
                nc.tensor.matmul(
                    psum_o[:, :], lhsT=UUb[:, :],
                    rhs=hsT_all[:, b * JB : (b + 1) * JB],
                    start=True, stop=False,
                )
                nc.tensor.matmul(
                    psum_o[:, :], lhsT=GT[:, :],
                    rhs=mhTd[:, b * JB : (b + 1) * JB],
                    start=False, stop=True,
                )
                nc.scalar.copy(out_t[b][:, :], psum_o[:, :])
                eng = nc.scalar if b == B - 1 else nc.sync
                eng.dma_start(out=out[b], in_=out_t[b][:, :])

            pdeg = emit_prep1()
            s1(0)
            emit_deg(pdeg)
            emit_mh_prep()
            # steady state: s1(t), s3(t-2), s2(t-1); s3 before s2 keeps the
            # ACT FIFO from blocking an output copy behind a me copy that
            # waits on a later-batch psum
            for t in range(1, B + 2):
                if t < B:
                    s1(t)
                if t <= B:
                    s2(t - 1)
                if t >= 2:
                    s3(t - 2)

    nc.compile()
    return nc


def _get_program():
    if "nc" not in _CACHE:
        _CACHE["nc"] = _build_program()
    return _CACHE["nc"]


def _make_in_maps(h, edge_attr, adj, W_w, W_b, U_w, U_b):
    import ml_dtypes

    bf = ml_dtypes.bfloat16
    h = np.asarray(h, dtype=np.float32)
    # host pre-cast to bf16: the kernel's first use of edge/h/hs rounds to
    # bf16 anyway (mask is 0/1), so results are bit-identical and the HBM
    # stream halves.
    edge_bf = np.asarray(edge_attr, dtype=np.float32).astype(bf)
    adj = np.asarray(adj, dtype=np.int8)
    W_w = np.ascontiguousarray(np.asarray(W_w, dtype=np.float32).astype(bf))
    W_b = np.ascontiguousarray(
        np.asarray(W_b, dtype=np.float32).astype(bf)
    ).reshape(1, H)
    U_w = np.ascontiguousarray(np.asarray(U_w, dtype=np.float32).astype(bf))
    U_b = np.ascontiguousarray(
        np.asarray(U_b, dtype=np.float32).astype(bf)
    ).reshape(1, H)

    # pre-arrange h to q-major [p, (q b f)] with i = 4p + q: matches the mh
    # matmul rhs layout directly, per-partition contiguous in DRAM
    h_bf = h.astype(bf)
    h_pre = np.ascontiguousarray(
        h_bf.reshape(B, 128, CH, H)
        .transpose(1, 2, 0, 3)
        .reshape(128, CH * B * H)
    )

    in_maps = []
    for c in range(NCORES):
        j0 = c * JB
        adj_c = adj[:, :, j0 : j0 + JB]  # [B, N, JB]
        adj_pre = np.ascontiguousarray(
            adj_c.reshape(B, 128, CH, JB)
            .transpose(1, 0, 2, 3)
            .reshape(128, B * CH * JB)
        )
        hs_pre = np.ascontiguousarray(
            h_bf[:, j0 : j0 + JB, :].transpose(1, 0, 2).reshape(JB, B * H)
        )
        in_maps.append(
            {
                "edge": np.ascontiguousarray(edge_bf[:, :, j0 : j0 + JB, :]),
                "adjs": adj_pre,
                "h": h_pre,
                "hs": hs_pre,
                "Ww": W_w,
                "Wb": W_b,
                "Uw": U_w,
                "Ub": U_b,
            }
        )
    return in_maps


def _install_ntff_hook():
    """The agent image lacks antenv.axon_hooks; synthesize it so trace=True
    can reach the libaxon NTFF profiling entry points."""
    import sys
    import types

    try:
        from antenv.axon_hooks import get_axon_ntff_profile_hook  # noqa: F401

        return
    except ImportError:
        pass
    import antenv

    mod = types.ModuleType("antenv.axon_hooks")
    _h = [None]
    mod.set_axon_ntff_profile_hook = lambda hook: _h.__setitem__(0, hook)
    mod.get_axon_ntff_profile_hook = lambda: _h[0]
    sys.modules["antenv.axon_hooks"] = mod
    antenv.axon_hooks = mod
    try:
        from trn_agent_boot.trn_boot import _ntff_profile_via_ctypes

        mod.set_axon_ntff_profile_hook(
            _ntff_profile_via_ctypes("/opt/axon/libaxon_pjrt.so")
        )
    except Exception:
        pass
    # avoid the bucket upload (no bucket in this container)
    import concourse.bass_utils as bu

    bu.upload_artifacts = lambda tmpdir: str(tmpdir)


def run(h, edge_attr, adj, W_w, W_b, U_w, U_b, trace=False, trace_cores=None):
    """Run the kernel; returns (output, BassKernelResults)."""
    _ensure_path()
    if trace:
        _install_ntff_hook()
    from concourse.bass_utils import run_bass_kernel_spmd

    nc = _get_program()
    in_maps = _make_in_maps(h, edge_attr, adj, W_w, W_b, U_w, U_b)
    kw = {}
    if trace:
        kw = {"trace": True, "trace_cores": trace_cores or [0]}
    res = run_bass_kernel_spmd(nc, in_maps, list(range(NCORES)), **kw)
    outs = [res.results[c]["out"].transpose(0, 2, 1) for c in range(NCORES)]
    full = np.concatenate(outs, axis=1)  # [B, N, H]
    return full, res


def kernel(h, edge_attr, adj, W_w, W_b, U_w, U_b):
    full, _ = run(h, edge_attr, adj, W_w, W_b, U_w, U_b)
    return full


# revision 68
# speedup vs baseline: 1.0173x; 1.0173x over previous
"""DMPNN layer kernel for 8 Trainium2 NeuronCores.

Sharding: data-parallel over destination nodes j (dim 2 of edge_attr/adj,
dim 1 of the output). Each core gets a 64-column j-slice of edge_attr/adj,
the full h (needed because messages sum over all source nodes i), and the
small weights replicated. The batch-global mask (adj.sum(0) > 0) only needs
the core's own j-slice of adj over the full batch, so no collective at all.

Math per core (j in its 64-column slice, source nodes i = 4p + q):
  mask[i,j]   = max_b adj[b,i,j]                    (adj is 0/1)
  deg[j]      = sum_i mask[i,j]
  mh[b,j,f]   = sum_i mask[i,j] h[b,i,f]
  me[b,j,e]   = sum_i mask[i,j] edge[b,i,j,e]
  out[b,j,o]  = sum_k G[o,k] x[b,k,j] + sum_f U[o,f] h[b,j,f] + ub[o]
     where G = U @ [Wh | wb | We]  (fused on device) and
     x[b] = [mh[b]^T; deg; me[b]^T]  (73 rows).

Design notes (measured on this part): DVE bf16 hits the 2x packed mode
only for fully-contiguous step-1 APs (broadcast/strided kill it); the
read-write bubble makes few/large ops win; DMA queues sharing HBM with
the edge stream starve ~30:1 when their descriptors are small, so every
input is host-pre-arranged per-partition-contiguous and loaded before
the stream; host pre-casts edge/h/hs/weights to bf16 and adj to int8 --
bit-identical results (the kernel's first op on each rounds to bf16
anyway; mask is 0/1) at half the HBM bytes.
 - sync queue: adjA, h, then the 4MiB bf16 edge stream (b7 in halves).
   scalar queue: adjB, ident, weights, hs, then b7's me move + store.
   gpsimd/SWDGE: steady-state me moves. Out stores ride sync.
 - mask: 3-op pairwise max tree; the two big levels run as bitwise-OR
   on int32-packed int8; pre-expanded over e to a contiguous bf16
   [128, q*j*e] tile so the per-batch multiply keeps the 2x mode.
 - mh for ALL batches via 4 accumulating matmuls (mask chunk stationary,
   h for all 8 batches as a 512-wide rhs), then one [64,64] PE transpose
   + copy per batch into the stacked rhs; deg via a ones-lhsT matmul
   folded on gpsimd.
 - per batch: contiguous bf16 DVE multiply -> contiguous half-fold ->
   (even b / b7) second fold + ONE e-major matmul, (odd b) two
   accumulating e-major matmuls -- alternation balances DVE vs PE and
   keeps the tail's PE queue short; psum_e lands me^T-flat and an
   8-descriptor move inserts it into the stacked rhs.
 - s3: ONE matmul pair [U^T;ub]x[hsT;1] + G^T x mhTd per batch, where
   G = U @ [Wh|wb|We] is fused on device once (msg never materializes).
"""

import numpy as np


def _ensure_path():
    try:
        import concourse.bass  # noqa: F401
    except ImportError:
        import sys

        for p in ("/opt/trn_rl_repo", "/root/.axon_site/_ro/trn_rl_repo"):
            if p not in sys.path:
                sys.path.insert(0, p)


B, N, H, E = 8, 512, 64, 8
NCORES = 8
JB = N // NCORES  # 64 destination columns per core
CH = N // 128  # 4 source-node sub-chunks (i = 4p + q)


_CACHE = {}


def _build_program():
    _ensure_path()
    import concourse.bacc as bacc
    import concourse.mybir as mybir
    import concourse.tile as tile

    dt = mybir.dt
    f32 = dt.float32
    bf16 = dt.bfloat16
    i32 = dt.int32
    Alu = mybir.AluOpType

    import ml_dtypes

    nc = bacc.Bacc("TRN2", debug=False, num_devices=NCORES)

    i8 = dt.int8
    # edge/h/hs are pre-cast to bf16 on the host: the kernel's first op on
    # them is a bf16-rounding multiply/matmul anyway (mask is 0/1), so the
    # result is bit-identical while the HBM stream halves to ~4.9 MiB.
    # adj holds 0/1 -> int8. All are pre-arranged per-partition-contiguous
    # (1KB-run descriptors measured only ~75 GB/s; 4-8KB runs go full rate).
    edge = nc.dram_tensor("edge", [B, N, JB, E], bf16, kind="ExternalInput").ap()
    adjs = nc.dram_tensor(
        "adjs", [128, B * (N // 128) * JB], i8, kind="ExternalInput"
    ).ap()
    h = nc.dram_tensor(
        "h", [128, (N // 128) * B * H], bf16, kind="ExternalInput"
    ).ap()
    hs = nc.dram_tensor("hs", [JB, B * H], bf16, kind="ExternalInput").ap()
    # weights host-cast to bf16 (they are rounded to bf16 on-chip anyway)
    Ww = nc.dram_tensor("Ww", [H, H + E], bf16, kind="ExternalInput").ap()
    Wb = nc.dram_tensor("Wb", [1, H], bf16, kind="ExternalInput").ap()
    Uw = nc.dram_tensor("Uw", [H, H], bf16, kind="ExternalInput").ap()
    Ub = nc.dram_tensor("Ub", [1, H], bf16, kind="ExternalInput").ap()
    out = nc.dram_tensor("out", [B, H, JB], f32, kind="ExternalOutput").ap()

    ident_d = nc.inline_tensor(
        np.eye(64).astype(ml_dtypes.bfloat16), "ident"
    )

    KM = H + 1 + E  # 73 contraction rows of the fused message matmul
    KU = H + 1  # 65 contraction rows of the base output matmul
    QJ = CH * JB  # 256
    QJE = CH * JB * E  # 2048
    JE = JB * E  # 512

    with tile.TileContext(nc) as tc:
        with (
            tc.tile_pool(name="const", bufs=1) as cpool,
            tc.tile_pool(name="masked", bufs=3) as mpool,
            tc.tile_pool(name="acc", bufs=3) as apool,
            tc.tile_pool(name="small", bufs=4) as spool,
            tc.tile_pool(name="pe", bufs=2, space="PSUM") as ppool_e,
            tc.tile_pool(name="pmh", bufs=1, space="PSUM") as ppool_mh,
            tc.tile_pool(name="ptr", bufs=2, space="PSUM") as ppool_tr,
            tc.tile_pool(name="pbase", bufs=2, space="PSUM") as ppool_base,
        ):
            # ---------------- DMA issue plan ----------------
            # All small/strided inputs must land BEFORE the edge stream:
            # the 8 KiB-descriptor edge DMAs starve any concurrent queue
            # down to ~1/30th bandwidth share.
            # sync queue: adjA(b0..3), h, then the edge stream.
            # scalar queue: adjB, ident, weights, hs, then per-batch smalls.
            adj_sb = cpool.tile([128, B * QJ], i8)
            nc.sync.dma_start(out=adj_sb[:, 0 : 4 * QJ], in_=adjs[:, 0 : 4 * QJ])
            nc.scalar.dma_start(out=adj_sb[:, 4 * QJ :], in_=adjs[:, 4 * QJ :])

            ident_bf = cpool.tile([64, 64], bf16)
            nc.scalar.dma_start(out=ident_bf[:, :], in_=ident_d.ap()[:, :])
            Ww_sb = cpool.tile([H, H + E], bf16)
            nc.scalar.dma_start(out=Ww_sb[:, :], in_=Ww[:, :])
            Uw_sb = cpool.tile([H, H], bf16)
            nc.scalar.dma_start(out=Uw_sb[:, :], in_=Uw[:, :])
            wb_sb = cpool.tile([1, H], bf16)
            nc.scalar.dma_start(out=wb_sb[:, :], in_=Wb[:, :])
            ub_sb = cpool.tile([1, H], bf16)
            nc.scalar.dma_start(out=ub_sb[:, :], in_=Ub[:, :])
            hs_all = cpool.tile([JB, B * H], bf16)
            nc.scalar.dma_start(out=hs_all[:, :], in_=hs[:, :])

            # h arrives pre-cast and already q-major: [p, (q b f)]
            h_bf2 = cpool.tile([128, B * CH * H], bf16)
            nc.sync.dma_start(out=h_bf2[:, :], in_=h[:, :])

            # the 4 MiB bf16 edge stream on sync; contiguous 4 KiB/partition
            edge_t = [
                cpool.tile([128, QJE], bf16, name=f"edge{b}") for b in range(B)
            ]
            EH = QJE // 2
            for b in range(B):
                src = edge[b].rearrange("(p q) j e -> p (q j e)", q=CH)
                if b == B - 1:
                    # halves: the tail multiply starts on the first half
                    nc.sync.dma_start(out=edge_t[b][:, 0:EH], in_=src[:, 0:EH])
                    nc.sync.dma_start(out=edge_t[b][:, EH:], in_=src[:, EH:])
                else:
                    nc.sync.dma_start(out=edge_t[b][:, :], in_=src)

            # ---- constants ----
            ones_bf = cpool.tile([128, 1], bf16)
            nc.vector.memset(ones_bf[:, :], 1.0)

            # ---- mask: 3-op pairwise max tree on contiguous halves; the
            # two big levels run as bitwise-OR on int32-packed int8 (0/1
            # values), quartering the DVE element count ----
            adj32 = adj_sb[:, :].bitcast(i32)  # [128, 512] packed words
            mt0 = cpool.tile([128, QJ], i32, name="mt0")
            nc.vector.tensor_tensor(
                mt0[:, :], adj32[:, 0:QJ], adj32[:, QJ:], Alu.bitwise_or
            )
            mt1 = cpool.tile([128, QJ // 2], i32, name="mt1")
            nc.vector.tensor_tensor(
                mt1[:, :], mt0[:, 0 : QJ // 2], mt0[:, QJ // 2 :], Alu.bitwise_or
            )
            mt1_8 = mt1[:, :].bitcast(i8)  # [128, 2*QJ] bytes
            mask_f = cpool.tile([128, QJ], f32)
            nc.vector.tensor_tensor(
                mask_f[:, :], mt1_8[:, 0:QJ], mt1_8[:, QJ:], Alu.max
            )
            mask_bf = cpool.tile([128, QJ], bf16)
            nc.vector.tensor_copy(mask_bf[:, :], mask_f[:, :])

            # pre-expanded bf16 mask over e: contiguous per-batch multiply
            mask_x = cpool.tile([128, QJE], bf16)
            nc.vector.tensor_copy(
                mask_x.rearrange("p (q j e) -> p q j e", q=CH, j=JB),
                mask_f.rearrange("p (q j) -> p q j", q=CH).broadcast_to(
                    [128, CH, JB, E]
                ),
            )


            # ---- stationary operands built on-chip (no DMA moves) ----
            A_bf = cpool.tile([H, KM], bf16)  # [Wh | wb | We] (cols)
            UUb = cpool.tile([KU, H], bf16)  # [U^T; ub]
            GT = cpool.tile([KM, H], bf16)  # (U @ A)^T
            hsT_all = cpool.tile([KU, B * JB], bf16)  # [hsT; ones] per b
            nc.vector.memset(hsT_all[H : H + 1, :], 1.0)
            # stacked rhs for all batches: [mhT; deg; me^T], b-major cols
            mhTd = cpool.tile([KM, B * JB], bf16)

            def emit_prep1():
                # A = [Wh | wb | We]
                pwb = ppool_tr.tile([H, 1], bf16, tag="t", name="pwb")
                nc.tensor.transpose(pwb[:, :], wb_sb[:, :], ident_bf[0:1, 0:1])
                puw = ppool_tr.tile([H, H], bf16, tag="t", name="puw")
                nc.tensor.transpose(puw[:, :], Uw_sb[:, :], ident_bf[0:H, 0:H])
                nc.scalar.copy(A_bf[:, 0:H], Ww_sb[:, 0:H])
                nc.scalar.copy(A_bf[:, H + 1 : KM], Ww_sb[:, H : H + E])
                nc.scalar.copy(A_bf[:, H : H + 1], pwb[:, :])
                nc.scalar.copy(UUb[0:H, :], puw[:, :])
                nc.scalar.copy(UUb[H : H + 1, :], ub_sb[:, :])

                # hsT per batch via PE transposes (hs arrives bf16)
                for b in range(B):
                    pht = ppool_tr.tile([H, JB], bf16, tag="t", name="pht")
                    nc.tensor.transpose(
                        pht[:, :], hs_all[:, b * H : (b + 1) * H], ident_bf[:, :]
                    )
                    nc.scalar.copy(hsT_all[0:H, b * JB : (b + 1) * JB], pht[:, :])

                # deg row matmul (folds emitted after s1(0) on DVE)
                pdeg = ppool_e.tile([1, QJ], f32, tag="e", name="pdeg")
                nc.tensor.matmul(
                    pdeg[:, :], lhsT=ones_bf[:, :], rhs=mask_bf[:, :],
                    start=True, stop=True,
                )

                # GT = (U @ A)^T = A^T U^T : lhsT=A [o,k], rhs=U^T [o,o']
                pg = ppool_mh.tile([KM, H], f32, name="pg")
                nc.tensor.matmul(
                    pg[:, :], lhsT=A_bf[:, :], rhs=UUb[0:H, :],
                    start=True, stop=True,
                )
                nc.scalar.copy(GT[:, :], pg[:, :])
                return pdeg

            def emit_deg(pdeg):
                # deg row: psum fold x2 + broadcast into mhTd row H
                # (all on the otherwise-idle gpsimd engine)
                dg0 = spool.tile([1, QJ], f32, name="dg0")
                nc.scalar.copy(dg0[:, :], pdeg[:, :])
                dg1 = spool.tile([1, 2 * JB], f32, name="dg1")
                nc.gpsimd.tensor_tensor(
                    dg1[:, :], dg0[:, 0 : 2 * JB], dg0[:, 2 * JB : QJ], Alu.add
                )
                dg2 = spool.tile([1, JB], bf16, name="dg2")
                nc.gpsimd.tensor_tensor(
                    dg2[:, :], dg1[:, 0:JB], dg1[:, JB : 2 * JB], Alu.add
                )
                nc.gpsimd.tensor_copy(
                    mhTd[H : H + 1, :].rearrange("p (b j) -> p b j", b=B),
                    dg2.rearrange("p (o j) -> p o j", o=1).broadcast_to(
                        [1, B, JB]
                    ),
                )

            def emit_mh_prep():
                # mh for ALL batches: mask chunk stationary, h 512-wide rhs
                pmh = ppool_mh.tile([JB, B * H], f32, name="pmh")
                for c in range(CH):
                    nc.tensor.matmul(
                        pmh[:, :],
                        lhsT=mask_bf[:, c * JB : (c + 1) * JB],
                        rhs=h_bf2[:, c * B * H : (c + 1) * B * H],
                        start=(c == 0),
                        stop=(c == CH - 1),
                    )
                mh_sb = cpool.tile([JB, B * H], bf16, name="mh_sb")
                nc.scalar.copy(mh_sb[:, :], pmh[:, :])



                # mhT per batch via PE transposes into the stacked rhs
                for b in range(B):
                    pmt = ppool_tr.tile([JB, H], bf16, tag="t", name="pmt")
                    nc.tensor.transpose(
                        pmt[:, :], mh_sb[:, b * H : (b + 1) * H], ident_bf[:, :]
                    )
                    nc.scalar.copy(mhTd[0:H, b * JB : (b + 1) * JB], pmt[:, :])

            # ---------------- per-batch software pipeline ----------------
            st = [dict() for _ in range(B)]

            def s1_mult(b):
                masked = mpool.tile([128, QJE], bf16, name="masked")
                # contiguous multiply (pre-expanded mask, no broadcast AP);
                # the tail batch multiplies per half-DMA to overlap arrival
                if b == B - 1:
                    for half in range(2):
                        nc.vector.tensor_tensor(
                            out=masked[:, half * EH : (half + 1) * EH],
                            in0=edge_t[b][:, half * EH : (half + 1) * EH],
                            in1=mask_x[:, half * EH : (half + 1) * EH],
                            op=Alu.mult,
                        )
                else:
                    nc.vector.tensor_tensor(
                        out=masked[:, :], in0=edge_t[b][:, :], in1=mask_x[:, :],
                        op=Alu.mult,
                    )
                st[b]["masked"] = masked

            def s1_fold(b):
                masked = st[b]["masked"]
                psum_e = ppool_e.tile([1, JE], f32, tag="e", name="psum_e")
                # contiguous half-fold: (q0+q2 | q1+q3)
                acc = apool.tile([128, EH], bf16, name="acc")
                nc.vector.tensor_tensor(
                    out=acc[:, :], in0=masked[:, 0:EH], in1=masked[:, EH:],
                    op=Alu.add,
                )
                # alternate fold depth to balance DVE vs PE; the tail batch
                # takes the short-DVE-chain path
                if b % 2 == 0 or b == B - 1:
                    asum = apool.tile([128, JE], bf16, name="asum")
                    nc.vector.tensor_tensor(
                        out=asum[:, :], in0=acc[:, 0:JE], in1=acc[:, JE:],
                        op=Alu.add,
                    )
                    nc.tensor.matmul(
                        psum_e[:, :],
                        lhsT=ones_bf[:, :],
                        rhs=asum.rearrange("p (j e) -> p e j", e=E),
                        start=True, stop=True,
                    )
                else:
                    for half in range(2):
                        nc.tensor.matmul(
                            psum_e[:, :],
                            lhsT=ones_bf[:, :],
                            rhs=acc[:, half * JE : (half + 1) * JE].rearrange(
                                "p (j e) -> p e j", e=E
                            ),
                            start=(half == 0),
                            stop=(half == 1),
                        )
                st[b]["psum_e"] = psum_e

            def s1(b):
                s1_mult(b)
                s1_fold(b)

            # dedicated output staging (stores ride the sync queue, which
            # drains behind the edge stream -- fine, they are terminal)
            out_t = [spool.tile([H, JB], f32, name=f"out{b}") for b in range(B)]

            def s2(b):
                # me^T rows into the stacked rhs (8-descriptor move; SWDGE
                # for steady-state, HWDGE for the latency-critical tail)
                d = st[b]
                me_sb = spool.tile([1, JE], bf16, name="me_sb")
                nc.scalar.copy(out=me_sb[:, :], in_=d["psum_e"][:, :])
                eng = nc.scalar if b == B - 1 else nc.gpsimd
                eng.dma_start(
                    out=mhTd[H + 1 : KM, b * JB : (b + 1) * JB],
                    in_=me_sb.rearrange("p (e j) -> p e j", e=E),
                )

            def s3(b):
                # out^T[b] = U^T-part (hsT) + G^T-part (mhTd), one psum
                psum_o = ppool_base.tile([H, JB], f32, name="psum_o")
                nc.tensor.matmul(
                    psum_o[:, :], lhsT=UUb[:, :],
                    rhs=hsT_all[:, b * JB : (b + 1) * JB],
                    start=True, stop=False,
                )
                nc.tensor.matmul(
                    psum_o[:, :], lhsT=GT[:, :],
                    rhs=mhTd[:, b * JB : (b + 1) * JB],
                    start=False, stop=True,
                )
                nc.scalar.copy(out_t[b][:, :], psum_o[:, :])
                eng = nc.scalar if b == B - 1 else nc.sync
                eng.dma_start(out=out[b], in_=out_t[b][:, :])

            pdeg = emit_prep1()
            s1(0)
            emit_deg(pdeg)
            emit_mh_prep()
            # steady state: s1(t), s3(t-2), s2(t-1); s3 before s2 keeps the
            # ACT FIFO from blocking an output copy behind a me copy that
            # waits on a later-batch psum
            for t in range(1, B + 2):
                if t < B:
                    s1(t)
                if t >= 2:
                    s3(t - 2)
                if t <= B:
                    s2(t - 1)

    nc.compile()
    return nc


def _get_program():
    if "nc" not in _CACHE:
        _CACHE["nc"] = _build_program()
    return _CACHE["nc"]


def _make_in_maps(h, edge_attr, adj, W_w, W_b, U_w, U_b):
    import ml_dtypes

    bf = ml_dtypes.bfloat16
    h = np.asarray(h, dtype=np.float32)
    # host pre-cast to bf16: the kernel's first use of edge/h/hs rounds to
    # bf16 anyway (mask is 0/1), so results are bit-identical and the HBM
    # stream halves.
    edge_bf = np.asarray(edge_attr, dtype=np.float32).astype(bf)
    adj = np.asarray(adj, dtype=np.int8)
    W_w = np.ascontiguousarray(np.asarray(W_w, dtype=np.float32).astype(bf))
    W_b = np.ascontiguousarray(
        np.asarray(W_b, dtype=np.float32).astype(bf)
    ).reshape(1, H)
    U_w = np.ascontiguousarray(np.asarray(U_w, dtype=np.float32).astype(bf))
    U_b = np.ascontiguousarray(
        np.asarray(U_b, dtype=np.float32).astype(bf)
    ).reshape(1, H)

    # pre-arrange h to q-major [p, (q b f)] with i = 4p + q: matches the mh
    # matmul rhs layout directly, per-partition contiguous in DRAM
    h_bf = h.astype(bf)
    h_pre = np.ascontiguousarray(
        h_bf.reshape(B, 128, CH, H)
        .transpose(1, 2, 0, 3)
        .reshape(128, CH * B * H)
    )

    in_maps = []
    for c in range(NCORES):
        j0 = c * JB
        adj_c = adj[:, :, j0 : j0 + JB]  # [B, N, JB]
        adj_pre = np.ascontiguousarray(
            adj_c.reshape(B, 128, CH, JB)
            .transpose(1, 0, 2, 3)
            .reshape(128, B * CH * JB)
        )
        hs_pre = np.ascontiguousarray(
            h_bf[:, j0 : j0 + JB, :].transpose(1, 0, 2).reshape(JB, B * H)
        )
        in_maps.append(
            {
                "edge": np.ascontiguousarray(edge_bf[:, :, j0 : j0 + JB, :]),
                "adjs": adj_pre,
                "h": h_pre,
                "hs": hs_pre,
                "Ww": W_w,
                "Wb": W_b,
                "Uw": U_w,
                "Ub": U_b,
            }
        )
    return in_maps


def _install_ntff_hook():
    """The agent image lacks antenv.axon_hooks; synthesize it so trace=True
    can reach the libaxon NTFF profiling entry points."""
    import sys
    import types

    try:
        from antenv.axon_hooks import get_axon_ntff_profile_hook  # noqa: F401

        return
    except ImportError:
        pass
    import antenv

    mod = types.ModuleType("antenv.axon_hooks")
    _h = [None]
    mod.set_axon_ntff_profile_hook = lambda hook: _h.__setitem__(0, hook)
    mod.get_axon_ntff_profile_hook = lambda: _h[0]
    sys.modules["antenv.axon_hooks"] = mod
    antenv.axon_hooks = mod
    try:
        from trn_agent_boot.trn_boot import _ntff_profile_via_ctypes

        mod.set_axon_ntff_profile_hook(
            _ntff_profile_via_ctypes("/opt/axon/libaxon_pjrt.so")
        )
    except Exception:
        pass
    # avoid the bucket upload (no bucket in this container)
    import concourse.bass_utils as bu

    bu.upload_artifacts = lambda tmpdir: str(tmpdir)


def run(h, edge_attr, adj, W_w, W_b, U_w, U_b, trace=False, trace_cores=None):
    """Run the kernel; returns (output, BassKernelResults)."""
    _ensure_path()
    if trace:
        _install_ntff_hook()
    from concourse.bass_utils import run_bass_kernel_spmd

    nc = _get_program()
    in_maps = _make_in_maps(h, edge_attr, adj, W_w, W_b, U_w, U_b)
    kw = {}
    if trace:
        kw = {"trace": True, "trace_cores": trace_cores or [0]}
    res = run_bass_kernel_spmd(nc, in_maps, list(range(NCORES)), **kw)
    outs = [res.results[c]["out"].transpose(0, 2, 1) for c in range(NCORES)]
    full = np.concatenate(outs, axis=1)  # [B, N, H]
    return full, res


def kernel(h, edge_attr, adj, W_w, W_b, U_w, U_b):
    full, _ = run(h, edge_attr, adj, W_w, W_b, U_w, U_b)
    return full


# revision 69
# speedup vs baseline: 1.0341x; 1.0165x over previous
"""DMPNN layer kernel for 8 Trainium2 NeuronCores.

Sharding: data-parallel over destination nodes j (dim 2 of edge_attr/adj,
dim 1 of the output). Each core gets a 64-column j-slice of edge_attr/adj,
the full h (needed because messages sum over all source nodes i), and the
small weights replicated. The batch-global mask (adj.sum(0) > 0) only needs
the core's own j-slice of adj over the full batch, so no collective at all.

Math per core (j in its 64-column slice, source nodes i = 4p + q):
  mask[i,j]   = max_b adj[b,i,j]                    (adj is 0/1)
  deg[j]      = sum_i mask[i,j]
  mh[b,j,f]   = sum_i mask[i,j] h[b,i,f]
  me[b,j,e]   = sum_i mask[i,j] edge[b,i,j,e]
  out[b,j,o]  = sum_k G[o,k] x[b,k,j] + sum_f U[o,f] h[b,j,f] + ub[o]
     where G = U @ [Wh | wb | We]  (fused on device) and
     x[b] = [mh[b]^T; deg; me[b]^T]  (73 rows).

Design notes (measured on this part): DVE bf16 hits the 2x packed mode
only for fully-contiguous step-1 APs (broadcast/strided kill it); the
read-write bubble makes few/large ops win; DMA queues sharing HBM with
the edge stream starve ~30:1 when their descriptors are small, so every
input is host-pre-arranged per-partition-contiguous and loaded before
the stream; host pre-casts edge/h/hs/weights to bf16 and adj to int8 --
bit-identical results (the kernel's first op on each rounds to bf16
anyway; mask is 0/1) at half the HBM bytes.
 - sync queue: adjA, h, then the 4MiB bf16 edge stream (b7 in halves).
   scalar queue: adjB, ident, weights, hs, then b7's me move + store.
   gpsimd/SWDGE: steady-state me moves. Out stores ride sync.
 - mask: 3-op pairwise max tree; the two big levels run as bitwise-OR
   on int32-packed int8; pre-expanded over e to a contiguous bf16
   [128, q*j*e] tile so the per-batch multiply keeps the 2x mode.
 - mh for ALL batches via 4 accumulating matmuls (mask chunk stationary,
   h for all 8 batches as a 512-wide rhs), then one [64,64] PE transpose
   + copy per batch into the stacked rhs; deg via a ones-lhsT matmul
   folded on gpsimd.
 - per batch: contiguous bf16 DVE multiply -> contiguous half-fold ->
   (even b / b7) second fold + ONE e-major matmul, (odd b) two
   accumulating e-major matmuls -- alternation balances DVE vs PE and
   keeps the tail's PE queue short; psum_e lands me^T-flat and an
   8-descriptor move inserts it into the stacked rhs.
 - s3: ONE matmul pair [U^T;ub]x[hsT;1] + G^T x mhTd per batch, where
   G = U @ [Wh|wb|We] is fused on device once (msg never materializes).
"""

import numpy as np


def _ensure_path():
    try:
        import concourse.bass  # noqa: F401
    except ImportError:
        import sys

        for p in ("/opt/trn_rl_repo", "/root/.axon_site/_ro/trn_rl_repo"):
            if p not in sys.path:
                sys.path.insert(0, p)


B, N, H, E = 8, 512, 64, 8
NCORES = 8
JB = N // NCORES  # 64 destination columns per core
CH = N // 128  # 4 source-node sub-chunks (i = 4p + q)


_CACHE = {}


def _build_program():
    _ensure_path()
    import concourse.bacc as bacc
    import concourse.mybir as mybir
    import concourse.tile as tile

    dt = mybir.dt
    f32 = dt.float32
    bf16 = dt.bfloat16
    i32 = dt.int32
    Alu = mybir.AluOpType

    import ml_dtypes

    nc = bacc.Bacc("TRN2", debug=False, num_devices=NCORES)

    i8 = dt.int8
    # edge/h/hs are pre-cast to bf16 on the host: the kernel's first op on
    # them is a bf16-rounding multiply/matmul anyway (mask is 0/1), so the
    # result is bit-identical while the HBM stream halves to ~4.9 MiB.
    # adj holds 0/1 -> int8. All are pre-arranged per-partition-contiguous
    # (1KB-run descriptors measured only ~75 GB/s; 4-8KB runs go full rate).
    edge = nc.dram_tensor("edge", [B, N, JB, E], bf16, kind="ExternalInput").ap()
    adjs = nc.dram_tensor(
        "adjs", [128, B * (N // 128) * JB], i8, kind="ExternalInput"
    ).ap()
    h = nc.dram_tensor(
        "h", [128, (N // 128) * B * H], bf16, kind="ExternalInput"
    ).ap()
    hs = nc.dram_tensor("hs", [JB, B * H], bf16, kind="ExternalInput").ap()
    # weights host-cast to bf16 (they are rounded to bf16 on-chip anyway)
    Ww = nc.dram_tensor("Ww", [H, H + E], bf16, kind="ExternalInput").ap()
    Wb = nc.dram_tensor("Wb", [1, H], bf16, kind="ExternalInput").ap()
    Uw = nc.dram_tensor("Uw", [H, H], bf16, kind="ExternalInput").ap()
    Ub = nc.dram_tensor("Ub", [1, H], bf16, kind="ExternalInput").ap()
    out = nc.dram_tensor("out", [B, H, JB], f32, kind="ExternalOutput").ap()

    ident_d = nc.inline_tensor(
        np.eye(64).astype(ml_dtypes.bfloat16), "ident"
    )

    KM = H + 1 + E  # 73 contraction rows of the fused message matmul
    KU = H + 1  # 65 contraction rows of the base output matmul
    QJ = CH * JB  # 256
    QJE = CH * JB * E  # 2048
    JE = JB * E  # 512

    with tile.TileContext(nc) as tc:
        with (
            tc.tile_pool(name="const", bufs=1) as cpool,
            tc.tile_pool(name="masked", bufs=3) as mpool,
            tc.tile_pool(name="acc", bufs=3) as apool,
            tc.tile_pool(name="small", bufs=4) as spool,
            tc.tile_pool(name="pe", bufs=2, space="PSUM") as ppool_e,
            tc.tile_pool(name="pmh", bufs=1, space="PSUM") as ppool_mh,
            tc.tile_pool(name="ptr", bufs=2, space="PSUM") as ppool_tr,
            tc.tile_pool(name="pbase", bufs=2, space="PSUM") as ppool_base,
        ):
            # ---------------- DMA issue plan ----------------
            # All small/strided inputs must land BEFORE the edge stream:
            # the 8 KiB-descriptor edge DMAs starve any concurrent queue
            # down to ~1/30th bandwidth share.
            # sync queue: adjA(b0..3), h, then the edge stream.
            # scalar queue: adjB, ident, weights, hs, then per-batch smalls.
            # adj (int8, 0.25 MiB) rides the scalar queue alone so the sync
            # queue starts the edge stream one h-transfer earlier
            adj_sb = cpool.tile([128, B * QJ], i8)
            nc.scalar.dma_start(out=adj_sb[:, :], in_=adjs[:, :])

            ident_bf = cpool.tile([64, 64], bf16)
            nc.scalar.dma_start(out=ident_bf[:, :], in_=ident_d.ap()[:, :])
            Ww_sb = cpool.tile([H, H + E], bf16)
            nc.scalar.dma_start(out=Ww_sb[:, :], in_=Ww[:, :])
            Uw_sb = cpool.tile([H, H], bf16)
            nc.scalar.dma_start(out=Uw_sb[:, :], in_=Uw[:, :])
            wb_sb = cpool.tile([1, H], bf16)
            nc.scalar.dma_start(out=wb_sb[:, :], in_=Wb[:, :])
            ub_sb = cpool.tile([1, H], bf16)
            nc.scalar.dma_start(out=ub_sb[:, :], in_=Ub[:, :])
            hs_all = cpool.tile([JB, B * H], bf16)
            nc.scalar.dma_start(out=hs_all[:, :], in_=hs[:, :])

            # h arrives pre-cast and already q-major: [p, (q b f)]
            h_bf2 = cpool.tile([128, B * CH * H], bf16)
            nc.sync.dma_start(out=h_bf2[:, :], in_=h[:, :])

            # the 4 MiB bf16 edge stream on sync; contiguous 4 KiB/partition
            edge_t = [
                cpool.tile([128, QJE], bf16, name=f"edge{b}") for b in range(B)
            ]
            EH = QJE // 2
            for b in range(B):
                src = edge[b].rearrange("(p q) j e -> p (q j e)", q=CH)
                if b == B - 1:
                    # halves: the tail multiply starts on the first half
                    nc.sync.dma_start(out=edge_t[b][:, 0:EH], in_=src[:, 0:EH])
                    nc.sync.dma_start(out=edge_t[b][:, EH:], in_=src[:, EH:])
                else:
                    nc.sync.dma_start(out=edge_t[b][:, :], in_=src)

            # ---- constants ----
            ones_bf = cpool.tile([128, 1], bf16)
            nc.vector.memset(ones_bf[:, :], 1.0)

            # ---- mask: 3-op pairwise max tree on contiguous halves; the
            # two big levels run as bitwise-OR on int32-packed int8 (0/1
            # values), quartering the DVE element count ----
            adj32 = adj_sb[:, :].bitcast(i32)  # [128, 512] packed words
            mt0 = cpool.tile([128, QJ], i32, name="mt0")
            nc.vector.tensor_tensor(
                mt0[:, :], adj32[:, 0:QJ], adj32[:, QJ:], Alu.bitwise_or
            )
            mt1 = cpool.tile([128, QJ // 2], i32, name="mt1")
            nc.vector.tensor_tensor(
                mt1[:, :], mt0[:, 0 : QJ // 2], mt0[:, QJ // 2 :], Alu.bitwise_or
            )
            mt1_8 = mt1[:, :].bitcast(i8)  # [128, 2*QJ] bytes
            mask_f = cpool.tile([128, QJ], f32)
            nc.vector.tensor_tensor(
                mask_f[:, :], mt1_8[:, 0:QJ], mt1_8[:, QJ:], Alu.max
            )
            mask_bf = cpool.tile([128, QJ], bf16)
            nc.vector.tensor_copy(mask_bf[:, :], mask_f[:, :])

            # pre-expanded bf16 mask over e: contiguous per-batch multiply
            mask_x = cpool.tile([128, QJE], bf16)
            nc.vector.tensor_copy(
                mask_x.rearrange("p (q j e) -> p q j e", q=CH, j=JB),
                mask_f.rearrange("p (q j) -> p q j", q=CH).broadcast_to(
                    [128, CH, JB, E]
                ),
            )


            # ---- stationary operands built on-chip (no DMA moves) ----
            A_bf = cpool.tile([H, KM], bf16)  # [Wh | wb | We] (cols)
            UUb = cpool.tile([KU, H], bf16)  # [U^T; ub]
            GT = cpool.tile([KM, H], bf16)  # (U @ A)^T
            hsT_all = cpool.tile([KU, B * JB], bf16)  # [hsT; ones] per b
            nc.vector.memset(hsT_all[H : H + 1, :], 1.0)
            # stacked rhs for all batches: [mhT; deg; me^T], b-major cols
            mhTd = cpool.tile([KM, B * JB], bf16)

            def emit_prep1():
                # A = [Wh | wb | We]
                pwb = ppool_tr.tile([H, 1], bf16, tag="t", name="pwb")
                nc.tensor.transpose(pwb[:, :], wb_sb[:, :], ident_bf[0:1, 0:1])
                puw = ppool_tr.tile([H, H], bf16, tag="t", name="puw")
                nc.tensor.transpose(puw[:, :], Uw_sb[:, :], ident_bf[0:H, 0:H])
                nc.scalar.copy(A_bf[:, 0:H], Ww_sb[:, 0:H])
                nc.scalar.copy(A_bf[:, H + 1 : KM], Ww_sb[:, H : H + E])
                nc.scalar.copy(A_bf[:, H : H + 1], pwb[:, :])
                nc.scalar.copy(UUb[0:H, :], puw[:, :])
                nc.scalar.copy(UUb[H : H + 1, :], ub_sb[:, :])

                # hsT per batch via PE transposes (hs arrives bf16)
                for b in range(B):
                    pht = ppool_tr.tile([H, JB], bf16, tag="t", name="pht")
                    nc.tensor.transpose(
                        pht[:, :], hs_all[:, b * H : (b + 1) * H], ident_bf[:, :]
                    )
                    nc.scalar.copy(hsT_all[0:H, b * JB : (b + 1) * JB], pht[:, :])

                # deg row matmul (folds emitted after s1(0) on DVE)
                pdeg = ppool_e.tile([1, QJ], f32, tag="e", name="pdeg")
                nc.tensor.matmul(
                    pdeg[:, :], lhsT=ones_bf[:, :], rhs=mask_bf[:, :],
                    start=True, stop=True,
                )

                # GT = (U @ A)^T = A^T U^T : lhsT=A [o,k], rhs=U^T [o,o']
                pg = ppool_mh.tile([KM, H], f32, name="pg")
                nc.tensor.matmul(
                    pg[:, :], lhsT=A_bf[:, :], rhs=UUb[0:H, :],
                    start=True, stop=True,
                )
                nc.scalar.copy(GT[:, :], pg[:, :])
                return pdeg

            def emit_deg(pdeg):
                # deg row: psum fold x2 + broadcast into mhTd row H
                # (all on the otherwise-idle gpsimd engine)
                dg0 = spool.tile([1, QJ], f32, name="dg0")
                nc.scalar.copy(dg0[:, :], pdeg[:, :])
                dg1 = spool.tile([1, 2 * JB], f32, name="dg1")
                nc.gpsimd.tensor_tensor(
                    dg1[:, :], dg0[:, 0 : 2 * JB], dg0[:, 2 * JB : QJ], Alu.add
                )
                dg2 = spool.tile([1, JB], bf16, name="dg2")
                nc.gpsimd.tensor_tensor(
                    dg2[:, :], dg1[:, 0:JB], dg1[:, JB : 2 * JB], Alu.add
                )
                nc.gpsimd.tensor_copy(
                    mhTd[H : H + 1, :].rearrange("p (b j) -> p b j", b=B),
                    dg2.rearrange("p (o j) -> p o j", o=1).broadcast_to(
                        [1, B, JB]
                    ),
                )

            def emit_mh_prep():
                # mh for ALL batches: mask chunk stationary, h 512-wide rhs
                pmh = ppool_mh.tile([JB, B * H], f32, name="pmh")
                for c in range(CH):
                    nc.tensor.matmul(
                        pmh[:, :],
                        lhsT=mask_bf[:, c * JB : (c + 1) * JB],
                        rhs=h_bf2[:, c * B * H : (c + 1) * B * H],
                        start=(c == 0),
                        stop=(c == CH - 1),
                    )
                mh_sb = cpool.tile([JB, B * H], bf16, name="mh_sb")
                nc.scalar.copy(mh_sb[:, :], pmh[:, :])



                # mhT per batch via PE transposes into the stacked rhs
                for b in range(B):
                    pmt = ppool_tr.tile([JB, H], bf16, tag="t", name="pmt")
                    nc.tensor.transpose(
                        pmt[:, :], mh_sb[:, b * H : (b + 1) * H], ident_bf[:, :]
                    )
                    nc.scalar.copy(mhTd[0:H, b * JB : (b + 1) * JB], pmt[:, :])

            # ---------------- per-batch software pipeline ----------------
            st = [dict() for _ in range(B)]

            def s1_mult(b):
                masked = mpool.tile([128, QJE], bf16, name="masked")
                # contiguous multiply (pre-expanded mask, no broadcast AP);
                # the tail batch multiplies per half-DMA to overlap arrival
                if b == B - 1:
                    for half in range(2):
                        nc.vector.tensor_tensor(
                            out=masked[:, half * EH : (half + 1) * EH],
                            in0=edge_t[b][:, half * EH : (half + 1) * EH],
                            in1=mask_x[:, half * EH : (half + 1) * EH],
                            op=Alu.mult,
                        )
                else:
                    nc.vector.tensor_tensor(
                        out=masked[:, :], in0=edge_t[b][:, :], in1=mask_x[:, :],
                        op=Alu.mult,
                    )
                st[b]["masked"] = masked

            def s1_fold(b):
                masked = st[b]["masked"]
                psum_e = ppool_e.tile([1, JE], f32, tag="e", name="psum_e")
                # contiguous half-fold: (q0+q2 | q1+q3)
                acc = apool.tile([128, EH], bf16, name="acc")
                nc.vector.tensor_tensor(
                    out=acc[:, :], in0=masked[:, 0:EH], in1=masked[:, EH:],
                    op=Alu.add,
                )
                # alternate fold depth to balance DVE vs PE; the tail batch
                # takes the short-DVE-chain path
                if b % 2 == 0 or b == B - 1:
                    asum = apool.tile([128, JE], bf16, name="asum")
                    nc.vector.tensor_tensor(
                        out=asum[:, :], in0=acc[:, 0:JE], in1=acc[:, JE:],
                        op=Alu.add,
                    )
                    nc.tensor.matmul(
                        psum_e[:, :],
                        lhsT=ones_bf[:, :],
                        rhs=asum.rearrange("p (j e) -> p e j", e=E),
                        start=True, stop=True,
                    )
                else:
                    for half in range(2):
                        nc.tensor.matmul(
                            psum_e[:, :],
                            lhsT=ones_bf[:, :],
                            rhs=acc[:, half * JE : (half + 1) * JE].rearrange(
                                "p (j e) -> p e j", e=E
                            ),
                            start=(half == 0),
                            stop=(half == 1),
                        )
                st[b]["psum_e"] = psum_e

            def s1(b):
                s1_mult(b)
                s1_fold(b)

            # dedicated output staging (stores ride the sync queue, which
            # drains behind the edge stream -- fine, they are terminal)
            out_t = [spool.tile([H, JB], f32, name=f"out{b}") for b in range(B)]

            def s2(b):
                # me^T rows into the stacked rhs (8-descriptor move; SWDGE
                # for steady-state, HWDGE for the latency-critical tail)
                d = st[b]
                me_sb = spool.tile([1, JE], bf16, name="me_sb")
                nc.scalar.copy(out=me_sb[:, :], in_=d["psum_e"][:, :])
                eng = nc.scalar if b == B - 1 else nc.gpsimd
                eng.dma_start(
                    out=mhTd[H + 1 : KM, b * JB : (b + 1) * JB],
                    in_=me_sb.rearrange("p (e j) -> p e j", e=E),
                )

            def s3(b):
                # out^T[b] = U^T-part (hsT) + G^T-part (mhTd), one psum
                psum_o = ppool_base.tile([H, JB], f32, name="psum_o")
                nc.tensor.matmul(
                    psum_o[:, :], lhsT=UUb[:, :],
                    rhs=hsT_all[:, b * JB : (b + 1) * JB],
                    start=True, stop=False,
                )
                nc.tensor.matmul(
                    psum_o[:, :], lhsT=GT[:, :],
                    rhs=mhTd[:, b * JB : (b + 1) * JB],
                    start=False, stop=True,
                )
                nc.scalar.copy(out_t[b][:, :], psum_o[:, :])
                eng = nc.scalar if b == B - 1 else nc.sync
                eng.dma_start(out=out[b], in_=out_t[b][:, :])

            pdeg = emit_prep1()
            s1(0)
            emit_deg(pdeg)
            emit_mh_prep()
            # steady state: s1(t), s3(t-2), s2(t-1); s3 before s2 keeps the
            # ACT FIFO from blocking an output copy behind a me copy that
            # waits on a later-batch psum
            for t in range(1, B + 2):
                if t < B:
                    s1(t)
                if t >= 2:
                    s3(t - 2)
                if t <= B:
                    s2(t - 1)

    nc.compile()
    return nc


def _get_program():
    if "nc" not in _CACHE:
        _CACHE["nc"] = _build_program()
    return _CACHE["nc"]


def _make_in_maps(h, edge_attr, adj, W_w, W_b, U_w, U_b):
    import ml_dtypes

    bf = ml_dtypes.bfloat16
    h = np.asarray(h, dtype=np.float32)
    # host pre-cast to bf16: the kernel's first use of edge/h/hs rounds to
    # bf16 anyway (mask is 0/1), so results are bit-identical and the HBM
    # stream halves.
    edge_bf = np.asarray(edge_attr, dtype=np.float32).astype(bf)
    adj = np.asarray(adj, dtype=np.int8)
    W_w = np.ascontiguousarray(np.asarray(W_w, dtype=np.float32).astype(bf))
    W_b = np.ascontiguousarray(
        np.asarray(W_b, dtype=np.float32).astype(bf)
    ).reshape(1, H)
    U_w = np.ascontiguousarray(np.asarray(U_w, dtype=np.float32).astype(bf))
    U_b = np.ascontiguousarray(
        np.asarray(U_b, dtype=np.float32).astype(bf)
    ).reshape(1, H)

    # pre-arrange h to q-major [p, (q b f)] with i = 4p + q: matches the mh
    # matmul rhs layout directly, per-partition contiguous in DRAM
    h_bf = h.astype(bf)
    h_pre = np.ascontiguousarray(
        h_bf.reshape(B, 128, CH, H)
        .transpose(1, 2, 0, 3)
        .reshape(128, CH * B * H)
    )

    in_maps = []
    for c in range(NCORES):
        j0 = c * JB
        adj_c = adj[:, :, j0 : j0 + JB]  # [B, N, JB]
        adj_pre = np.ascontiguousarray(
            adj_c.reshape(B, 128, CH, JB)
            .transpose(1, 0, 2, 3)
            .reshape(128, B * CH * JB)
        )
        hs_pre = np.ascontiguousarray(
            h_bf[:, j0 : j0 + JB, :].transpose(1, 0, 2).reshape(JB, B * H)
        )
        in_maps.append(
            {
                "edge": np.ascontiguousarray(edge_bf[:, :, j0 : j0 + JB, :]),
                "adjs": adj_pre,
                "h": h_pre,
                "hs": hs_pre,
                "Ww": W_w,
                "Wb": W_b,
                "Uw": U_w,
                "Ub": U_b,
            }
        )
    return in_maps


def _install_ntff_hook():
    """The agent image lacks antenv.axon_hooks; synthesize it so trace=True
    can reach the libaxon NTFF profiling entry points."""
    import sys
    import types

    try:
        from antenv.axon_hooks import get_axon_ntff_profile_hook  # noqa: F401

        return
    except ImportError:
        pass
    import antenv

    mod = types.ModuleType("antenv.axon_hooks")
    _h = [None]
    mod.set_axon_ntff_profile_hook = lambda hook: _h.__setitem__(0, hook)
    mod.get_axon_ntff_profile_hook = lambda: _h[0]
    sys.modules["antenv.axon_hooks"] = mod
    antenv.axon_hooks = mod
    try:
        from trn_agent_boot.trn_boot import _ntff_profile_via_ctypes

        mod.set_axon_ntff_profile_hook(
            _ntff_profile_via_ctypes("/opt/axon/libaxon_pjrt.so")
        )
    except Exception:
        pass
    # avoid the bucket upload (no bucket in this container)
    import concourse.bass_utils as bu

    bu.upload_artifacts = lambda tmpdir: str(tmpdir)


def run(h, edge_attr, adj, W_w, W_b, U_w, U_b, trace=False, trace_cores=None):
    """Run the kernel; returns (output, BassKernelResults)."""
    _ensure_path()
    if trace:
        _install_ntff_hook()
    from concourse.bass_utils import run_bass_kernel_spmd

    nc = _get_program()
    in_maps = _make_in_maps(h, edge_attr, adj, W_w, W_b, U_w, U_b)
    kw = {}
    if trace:
        kw = {"trace": True, "trace_cores": trace_cores or [0]}
    res = run_bass_kernel_spmd(nc, in_maps, list(range(NCORES)), **kw)
    outs = [res.results[c]["out"].transpose(0, 2, 1) for c in range(NCORES)]
    full = np.concatenate(outs, axis=1)  # [B, N, H]
    return full, res


def kernel(h, edge_attr, adj, W_w, W_b, U_w, U_b):
    full, _ = run(h, edge_attr, adj, W_w, W_b, U_w, U_b)
    return full


# revision 70
# speedup vs baseline: 1.0479x; 1.0133x over previous
"""DMPNN layer kernel for 8 Trainium2 NeuronCores.

Sharding: data-parallel over destination nodes j (dim 2 of edge_attr/adj,
dim 1 of the output). Each core gets a 64-column j-slice of edge_attr/adj,
the full h (needed because messages sum over all source nodes i), and the
small weights replicated. The batch-global mask (adj.sum(0) > 0) only needs
the core's own j-slice of adj over the full batch, so no collective at all.

Math per core (j in its 64-column slice, source nodes i = 4p + q):
  mask[i,j]   = max_b adj[b,i,j]                    (adj is 0/1)
  deg[j]      = sum_i mask[i,j]
  mh[b,j,f]   = sum_i mask[i,j] h[b,i,f]
  me[b,j,e]   = sum_i mask[i,j] edge[b,i,j,e]
  out[b,j,o]  = sum_k G[o,k] x[b,k,j] + sum_f U[o,f] h[b,j,f] + ub[o]
     where G = U @ [Wh | wb | We]  (fused on device) and
     x[b] = [mh[b]^T; deg; me[b]^T]  (73 rows).

Design notes (measured on this part): DVE bf16 hits the 2x packed mode
only for fully-contiguous step-1 APs (broadcast/strided kill it); the
read-write bubble makes few/large ops win; DMA queues sharing HBM with
the edge stream starve ~30:1 when their descriptors are small, so every
input is host-pre-arranged per-partition-contiguous and loaded before
the stream; host pre-casts edge/h/hs/weights to bf16 and adj to int8 --
bit-identical results (the kernel's first op on each rounds to bf16
anyway; mask is 0/1) at half the HBM bytes.
 - sync queue: adjA, h, then the 4MiB bf16 edge stream (b7 in halves).
   scalar queue: adjB, ident, weights, hs, then b7's me move + store.
   gpsimd/SWDGE: steady-state me moves. Out stores ride sync.
 - mask: 3-op pairwise max tree; the two big levels run as bitwise-OR
   on int32-packed int8; pre-expanded over e to a contiguous bf16
   [128, q*j*e] tile so the per-batch multiply keeps the 2x mode.
 - mh for ALL batches via 4 accumulating matmuls (mask chunk stationary,
   h for all 8 batches as a 512-wide rhs), then one [64,64] PE transpose
   + copy per batch into the stacked rhs; deg via a ones-lhsT matmul
   folded on gpsimd.
 - per batch: contiguous bf16 DVE multiply -> contiguous half-fold ->
   (even b / b7) second fold + ONE e-major matmul, (odd b) two
   accumulating e-major matmuls -- alternation balances DVE vs PE and
   keeps the tail's PE queue short; psum_e lands me^T-flat and an
   8-descriptor move inserts it into the stacked rhs.
 - s3: ONE matmul pair [U^T;ub]x[hsT;1] + G^T x mhTd per batch, where
   G = U @ [Wh|wb|We] is fused on device once (msg never materializes).
"""

import numpy as np


def _ensure_path():
    try:
        import concourse.bass  # noqa: F401
    except ImportError:
        import sys

        for p in ("/opt/trn_rl_repo", "/root/.axon_site/_ro/trn_rl_repo"):
            if p not in sys.path:
                sys.path.insert(0, p)


B, N, H, E = 8, 512, 64, 8
NCORES = 8
JB = N // NCORES  # 64 destination columns per core
CH = N // 128  # 4 source-node sub-chunks (i = 4p + q)


_CACHE = {}


def _build_program():
    _ensure_path()
    import concourse.bacc as bacc
    import concourse.mybir as mybir
    import concourse.tile as tile

    dt = mybir.dt
    f32 = dt.float32
    bf16 = dt.bfloat16
    i32 = dt.int32
    Alu = mybir.AluOpType

    import ml_dtypes

    nc = bacc.Bacc("TRN2", debug=False, num_devices=NCORES)

    i8 = dt.int8
    # edge/h/hs are pre-cast to bf16 on the host: the kernel's first op on
    # them is a bf16-rounding multiply/matmul anyway (mask is 0/1), so the
    # result is bit-identical while the HBM stream halves to ~4.9 MiB.
    # adj holds 0/1 -> int8. All are pre-arranged per-partition-contiguous
    # (1KB-run descriptors measured only ~75 GB/s; 4-8KB runs go full rate).
    edge = nc.dram_tensor("edge", [B, N, JB, E], bf16, kind="ExternalInput").ap()
    adjs = nc.dram_tensor(
        "adjs", [128, B * (N // 128) * JB], i8, kind="ExternalInput"
    ).ap()
    h = nc.dram_tensor(
        "h", [128, (N // 128) * B * H], bf16, kind="ExternalInput"
    ).ap()
    hs = nc.dram_tensor("hs", [JB, B * H], bf16, kind="ExternalInput").ap()
    # weights host-cast to bf16 (they are rounded to bf16 on-chip anyway)
    Ww = nc.dram_tensor("Ww", [H, H + E], bf16, kind="ExternalInput").ap()
    Wb = nc.dram_tensor("Wb", [1, H], bf16, kind="ExternalInput").ap()
    Uw = nc.dram_tensor("Uw", [H, H], bf16, kind="ExternalInput").ap()
    Ub = nc.dram_tensor("Ub", [1, H], bf16, kind="ExternalInput").ap()
    out = nc.dram_tensor("out", [B, H, JB], f32, kind="ExternalOutput").ap()

    ident_d = nc.inline_tensor(
        np.eye(64).astype(ml_dtypes.bfloat16), "ident"
    )

    KM = H + 1 + E  # 73 contraction rows of the fused message matmul
    KU = H + 1  # 65 contraction rows of the base output matmul
    QJ = CH * JB  # 256
    QJE = CH * JB * E  # 2048
    JE = JB * E  # 512

    with tile.TileContext(nc) as tc:
        with (
            tc.tile_pool(name="const", bufs=1) as cpool,
            tc.tile_pool(name="masked", bufs=3) as mpool,
            tc.tile_pool(name="acc", bufs=3) as apool,
            tc.tile_pool(name="small", bufs=4) as spool,
            tc.tile_pool(name="pe", bufs=2, space="PSUM") as ppool_e,
            tc.tile_pool(name="pmh", bufs=1, space="PSUM") as ppool_mh,
            tc.tile_pool(name="ptr", bufs=2, space="PSUM") as ppool_tr,
            tc.tile_pool(name="pbase", bufs=2, space="PSUM") as ppool_base,
        ):
            # ---------------- DMA issue plan ----------------
            # All small/strided inputs must land BEFORE the edge stream:
            # the 8 KiB-descriptor edge DMAs starve any concurrent queue
            # down to ~1/30th bandwidth share.
            # sync queue: adjA(b0..3), h, then the edge stream.
            # scalar queue: adjB, ident, weights, hs, then per-batch smalls.
            # adj (int8, 0.25 MiB) rides the scalar queue alone so the sync
            # queue starts the edge stream one h-transfer earlier
            adj_sb = cpool.tile([128, B * QJ], i8)
            nc.scalar.dma_start(out=adj_sb[:, :], in_=adjs[:, :])

            ident_bf = cpool.tile([64, 64], bf16)
            nc.scalar.dma_start(out=ident_bf[:, :], in_=ident_d.ap()[:, :])
            Ww_sb = cpool.tile([H, H + E], bf16)
            nc.scalar.dma_start(out=Ww_sb[:, :], in_=Ww[:, :])
            Uw_sb = cpool.tile([H, H], bf16)
            nc.scalar.dma_start(out=Uw_sb[:, :], in_=Uw[:, :])
            wb_sb = cpool.tile([1, H], bf16)
            nc.scalar.dma_start(out=wb_sb[:, :], in_=Wb[:, :])
            ub_sb = cpool.tile([1, H], bf16)
            nc.scalar.dma_start(out=ub_sb[:, :], in_=Ub[:, :])
            hs_all = cpool.tile([JB, B * H], bf16)
            nc.scalar.dma_start(out=hs_all[:, :], in_=hs[:, :])

            # h arrives pre-cast and already q-major: [p, (q b f)]
            h_bf2 = cpool.tile([128, B * CH * H], bf16)
            nc.sync.dma_start(out=h_bf2[:, :], in_=h[:, :])

            # the 4 MiB bf16 edge stream on sync; contiguous 4 KiB/partition
            edge_t = [
                cpool.tile([128, QJE], bf16, name=f"edge{b}") for b in range(B)
            ]
            EH = QJE // 2
            for b in range(B):
                src = edge[b].rearrange("(p q) j e -> p (q j e)", q=CH)
                if b == B - 1:
                    # halves: the tail multiply starts on the first half
                    nc.sync.dma_start(out=edge_t[b][:, 0:EH], in_=src[:, 0:EH])
                    nc.sync.dma_start(out=edge_t[b][:, EH:], in_=src[:, EH:])
                else:
                    nc.sync.dma_start(out=edge_t[b][:, :], in_=src)

            # ---- constants ----
            ones_bf = cpool.tile([128, 1], bf16)
            nc.vector.memset(ones_bf[:, :], 1.0)

            # ---- mask: 3-op pairwise max tree on contiguous halves; the
            # two big levels run as bitwise-OR on int32-packed int8 (0/1
            # values), quartering the DVE element count ----
            adj32 = adj_sb[:, :].bitcast(i32)  # [128, 512] packed words
            mt0 = cpool.tile([128, QJ], i32, name="mt0")
            nc.vector.tensor_tensor(
                mt0[:, :], adj32[:, 0:QJ], adj32[:, QJ:], Alu.bitwise_or
            )
            mt1 = cpool.tile([128, QJ // 2], i32, name="mt1")
            nc.vector.tensor_tensor(
                mt1[:, :], mt0[:, 0 : QJ // 2], mt0[:, QJ // 2 :], Alu.bitwise_or
            )
            mt1_8 = mt1[:, :].bitcast(i8)  # [128, 2*QJ] bytes
            mask_f = cpool.tile([128, QJ], f32)
            nc.vector.tensor_tensor(
                mask_f[:, :], mt1_8[:, 0:QJ], mt1_8[:, QJ:], Alu.max
            )
            # pre-expanded bf16 mask over e (emitted before the mask_bf
            # cast: only the expand gates the first edge multiply)
            mask_x = cpool.tile([128, QJE], bf16)
            nc.vector.tensor_copy(
                mask_x.rearrange("p (q j e) -> p q j e", q=CH, j=JB),
                mask_f.rearrange("p (q j) -> p q j", q=CH).broadcast_to(
                    [128, CH, JB, E]
                ),
            )
            mask_bf = cpool.tile([128, QJ], bf16)
            nc.vector.tensor_copy(mask_bf[:, :], mask_f[:, :])


            # ---- stationary operands built on-chip (no DMA moves) ----
            A_bf = cpool.tile([H, KM], bf16)  # [Wh | wb | We] (cols)
            UUb = cpool.tile([KU, H], bf16)  # [U^T; ub]
            GT = cpool.tile([KM, H], bf16)  # (U @ A)^T
            hsT_all = cpool.tile([KU, B * JB], bf16)  # [hsT; ones] per b
            nc.vector.memset(hsT_all[H : H + 1, :], 1.0)
            # stacked rhs for all batches: [mhT; deg; me^T], b-major cols
            mhTd = cpool.tile([KM, B * JB], bf16)

            def emit_prep1():
                # A = [Wh | wb | We]
                pwb = ppool_tr.tile([H, 1], bf16, tag="t", name="pwb")
                nc.tensor.transpose(pwb[:, :], wb_sb[:, :], ident_bf[0:1, 0:1])
                puw = ppool_tr.tile([H, H], bf16, tag="t", name="puw")
                nc.tensor.transpose(puw[:, :], Uw_sb[:, :], ident_bf[0:H, 0:H])
                nc.scalar.copy(A_bf[:, 0:H], Ww_sb[:, 0:H])
                nc.scalar.copy(A_bf[:, H + 1 : KM], Ww_sb[:, H : H + E])
                nc.scalar.copy(A_bf[:, H : H + 1], pwb[:, :])
                nc.scalar.copy(UUb[0:H, :], puw[:, :])
                nc.scalar.copy(UUb[H : H + 1, :], ub_sb[:, :])

                # hsT per batch via PE transposes (hs arrives bf16)
                for b in range(B):
                    pht = ppool_tr.tile([H, JB], bf16, tag="t", name="pht")
                    nc.tensor.transpose(
                        pht[:, :], hs_all[:, b * H : (b + 1) * H], ident_bf[:, :]
                    )
                    nc.scalar.copy(hsT_all[0:H, b * JB : (b + 1) * JB], pht[:, :])

                # deg row matmul (folds emitted after s1(0) on DVE)
                pdeg = ppool_e.tile([1, QJ], f32, tag="e", name="pdeg")
                nc.tensor.matmul(
                    pdeg[:, :], lhsT=ones_bf[:, :], rhs=mask_bf[:, :],
                    start=True, stop=True,
                )

                # GT = (U @ A)^T = A^T U^T : lhsT=A [o,k], rhs=U^T [o,o']
                pg = ppool_mh.tile([KM, H], f32, name="pg")
                nc.tensor.matmul(
                    pg[:, :], lhsT=A_bf[:, :], rhs=UUb[0:H, :],
                    start=True, stop=True,
                )
                nc.scalar.copy(GT[:, :], pg[:, :])
                return pdeg

            def emit_deg(pdeg):
                # deg row: psum fold x2 + broadcast into mhTd row H
                # (all on the otherwise-idle gpsimd engine)
                dg0 = spool.tile([1, QJ], f32, name="dg0")
                nc.scalar.copy(dg0[:, :], pdeg[:, :])
                dg1 = spool.tile([1, 2 * JB], f32, name="dg1")
                nc.gpsimd.tensor_tensor(
                    dg1[:, :], dg0[:, 0 : 2 * JB], dg0[:, 2 * JB : QJ], Alu.add
                )
                dg2 = spool.tile([1, JB], bf16, name="dg2")
                nc.gpsimd.tensor_tensor(
                    dg2[:, :], dg1[:, 0:JB], dg1[:, JB : 2 * JB], Alu.add
                )
                nc.gpsimd.tensor_copy(
                    mhTd[H : H + 1, :].rearrange("p (b j) -> p b j", b=B),
                    dg2.rearrange("p (o j) -> p o j", o=1).broadcast_to(
                        [1, B, JB]
                    ),
                )

            def emit_mh_prep():
                # mh for ALL batches: mask chunk stationary, h 512-wide rhs
                pmh = ppool_mh.tile([JB, B * H], f32, name="pmh")
                for c in range(CH):
                    nc.tensor.matmul(
                        pmh[:, :],
                        lhsT=mask_bf[:, c * JB : (c + 1) * JB],
                        rhs=h_bf2[:, c * B * H : (c + 1) * B * H],
                        start=(c == 0),
                        stop=(c == CH - 1),
                    )
                mh_sb = cpool.tile([JB, B * H], bf16, name="mh_sb")
                nc.scalar.copy(mh_sb[:, :], pmh[:, :])



                # mhT per batch via PE transposes into the stacked rhs
                for b in range(B):
                    pmt = ppool_tr.tile([JB, H], bf16, tag="t", name="pmt")
                    nc.tensor.transpose(
                        pmt[:, :], mh_sb[:, b * H : (b + 1) * H], ident_bf[:, :]
                    )
                    nc.scalar.copy(mhTd[0:H, b * JB : (b + 1) * JB], pmt[:, :])

            # ---------------- per-batch software pipeline ----------------
            st = [dict() for _ in range(B)]

            def s1_mult(b):
                masked = mpool.tile([128, QJE], bf16, name="masked")
                # contiguous multiply (pre-expanded mask, no broadcast AP);
                # the tail batch multiplies per half-DMA to overlap arrival
                if b == B - 1:
                    for half in range(2):
                        nc.vector.tensor_tensor(
                            out=masked[:, half * EH : (half + 1) * EH],
                            in0=edge_t[b][:, half * EH : (half + 1) * EH],
                            in1=mask_x[:, half * EH : (half + 1) * EH],
                            op=Alu.mult,
                        )
                else:
                    nc.vector.tensor_tensor(
                        out=masked[:, :], in0=edge_t[b][:, :], in1=mask_x[:, :],
                        op=Alu.mult,
                    )
                st[b]["masked"] = masked

            def s1_fold(b):
                masked = st[b]["masked"]
                psum_e = ppool_e.tile([1, JE], f32, tag="e", name="psum_e")
                # contiguous half-fold: (q0+q2 | q1+q3)
                acc = apool.tile([128, EH], bf16, name="acc")
                nc.vector.tensor_tensor(
                    out=acc[:, :], in0=masked[:, 0:EH], in1=masked[:, EH:],
                    op=Alu.add,
                )
                # alternate fold depth to balance DVE vs PE; the tail batch
                # takes the short-DVE-chain path
                if b % 2 == 0 or b == B - 1:
                    asum = apool.tile([128, JE], bf16, name="asum")
                    nc.vector.tensor_tensor(
                        out=asum[:, :], in0=acc[:, 0:JE], in1=acc[:, JE:],
                        op=Alu.add,
                    )
                    nc.tensor.matmul(
                        psum_e[:, :],
                        lhsT=ones_bf[:, :],
                        rhs=asum.rearrange("p (j e) -> p e j", e=E),
                        start=True, stop=True,
                    )
                else:
                    for half in range(2):
                        nc.tensor.matmul(
                            psum_e[:, :],
                            lhsT=ones_bf[:, :],
                            rhs=acc[:, half * JE : (half + 1) * JE].rearrange(
                                "p (j e) -> p e j", e=E
                            ),
                            start=(half == 0),
                            stop=(half == 1),
                        )
                st[b]["psum_e"] = psum_e

            def s1(b):
                s1_mult(b)
                s1_fold(b)

            # dedicated output staging (stores ride the sync queue, which
            # drains behind the edge stream -- fine, they are terminal)
            out_t = [spool.tile([H, JB], f32, name=f"out{b}") for b in range(B)]

            def s2(b):
                # me^T rows into the stacked rhs (8-descriptor move; SWDGE
                # for steady-state, HWDGE for the latency-critical tail)
                d = st[b]
                me_sb = spool.tile([1, JE], bf16, name="me_sb")
                nc.scalar.copy(out=me_sb[:, :], in_=d["psum_e"][:, :])
                eng = nc.scalar if b == B - 1 else nc.gpsimd
                eng.dma_start(
                    out=mhTd[H + 1 : KM, b * JB : (b + 1) * JB],
                    in_=me_sb.rearrange("p (e j) -> p e j", e=E),
                )

            def s3(b):
                # out^T[b] = U^T-part (hsT) + G^T-part (mhTd), one psum
                psum_o = ppool_base.tile([H, JB], f32, name="psum_o")
                nc.tensor.matmul(
                    psum_o[:, :], lhsT=UUb[:, :],
                    rhs=hsT_all[:, b * JB : (b + 1) * JB],
                    start=True, stop=False,
                )
                nc.tensor.matmul(
                    psum_o[:, :], lhsT=GT[:, :],
                    rhs=mhTd[:, b * JB : (b + 1) * JB],
                    start=False, stop=True,
                )
                nc.scalar.copy(out_t[b][:, :], psum_o[:, :])
                eng = nc.scalar if b == B - 1 else nc.sync
                eng.dma_start(out=out[b], in_=out_t[b][:, :])

            pdeg = emit_prep1()
            s1(0)
            emit_deg(pdeg)
            emit_mh_prep()
            # steady state: s1(t), s3(t-2), s2(t-1); s3 before s2 keeps the
            # ACT FIFO from blocking an output copy behind a me copy that
            # waits on a later-batch psum
            for t in range(1, B + 2):
                if t < B:
                    s1(t)
                if t >= 2:
                    s3(t - 2)
                if t <= B:
                    s2(t - 1)

    nc.compile()
    return nc


def _get_program():
    if "nc" not in _CACHE:
        _CACHE["nc"] = _build_program()
    return _CACHE["nc"]


def _make_in_maps(h, edge_attr, adj, W_w, W_b, U_w, U_b):
    import ml_dtypes

    bf = ml_dtypes.bfloat16
    h = np.asarray(h, dtype=np.float32)
    # host pre-cast to bf16: the kernel's first use of edge/h/hs rounds to
    # bf16 anyway (mask is 0/1), so results are bit-identical and the HBM
    # stream halves.
    edge_bf = np.asarray(edge_attr, dtype=np.float32).astype(bf)
    adj = np.asarray(adj, dtype=np.int8)
    W_w = np.ascontiguousarray(np.asarray(W_w, dtype=np.float32).astype(bf))
    W_b = np.ascontiguousarray(
        np.asarray(W_b, dtype=np.float32).astype(bf)
    ).reshape(1, H)
    U_w = np.ascontiguousarray(np.asarray(U_w, dtype=np.float32).astype(bf))
    U_b = np.ascontiguousarray(
        np.asarray(U_b, dtype=np.float32).astype(bf)
    ).reshape(1, H)

    # pre-arrange h to q-major [p, (q b f)] with i = 4p + q: matches the mh
    # matmul rhs layout directly, per-partition contiguous in DRAM
    h_bf = h.astype(bf)
    h_pre = np.ascontiguousarray(
        h_bf.reshape(B, 128, CH, H)
        .transpose(1, 2, 0, 3)
        .reshape(128, CH * B * H)
    )

    in_maps = []
    for c in range(NCORES):
        j0 = c * JB
        adj_c = adj[:, :, j0 : j0 + JB]  # [B, N, JB]
        adj_pre = np.ascontiguousarray(
            adj_c.reshape(B, 128, CH, JB)
            .transpose(1, 0, 2, 3)
            .reshape(128, B * CH * JB)
        )
        hs_pre = np.ascontiguousarray(
            h_bf[:, j0 : j0 + JB, :].transpose(1, 0, 2).reshape(JB, B * H)
        )
        in_maps.append(
            {
                "edge": np.ascontiguousarray(edge_bf[:, :, j0 : j0 + JB, :]),
                "adjs": adj_pre,
                "h": h_pre,
                "hs": hs_pre,
                "Ww": W_w,
                "Wb": W_b,
                "Uw": U_w,
                "Ub": U_b,
            }
        )
    return in_maps


def _install_ntff_hook():
    """The agent image lacks antenv.axon_hooks; synthesize it so trace=True
    can reach the libaxon NTFF profiling entry points."""
    import sys
    import types

    try:
        from antenv.axon_hooks import get_axon_ntff_profile_hook  # noqa: F401

        return
    except ImportError:
        pass
    import antenv

    mod = types.ModuleType("antenv.axon_hooks")
    _h = [None]
    mod.set_axon_ntff_profile_hook = lambda hook: _h.__setitem__(0, hook)
    mod.get_axon_ntff_profile_hook = lambda: _h[0]
    sys.modules["antenv.axon_hooks"] = mod
    antenv.axon_hooks = mod
    try:
        from trn_agent_boot.trn_boot import _ntff_profile_via_ctypes

        mod.set_axon_ntff_profile_hook(
            _ntff_profile_via_ctypes("/opt/axon/libaxon_pjrt.so")
        )
    except Exception:
        pass
    # avoid the bucket upload (no bucket in this container)
    import concourse.bass_utils as bu

    bu.upload_artifacts = lambda tmpdir: str(tmpdir)


def run(h, edge_attr, adj, W_w, W_b, U_w, U_b, trace=False, trace_cores=None):
    """Run the kernel; returns (output, BassKernelResults)."""
    _ensure_path()
    if trace:
        _install_ntff_hook()
    from concourse.bass_utils import run_bass_kernel_spmd

    nc = _get_program()
    in_maps = _make_in_maps(h, edge_attr, adj, W_w, W_b, U_w, U_b)
    kw = {}
    if trace:
        kw = {"trace": True, "trace_cores": trace_cores or [0]}
    res = run_bass_kernel_spmd(nc, in_maps, list(range(NCORES)), **kw)
    outs = [res.results[c]["out"].transpose(0, 2, 1) for c in range(NCORES)]
    full = np.concatenate(outs, axis=1)  # [B, N, H]
    return full, res


def kernel(h, edge_attr, adj, W_w, W_b, U_w, U_b):
    full, _ = run(h, edge_attr, adj, W_w, W_b, U_w, U_b)
    return full
